# revision 30
# baseline (speedup 1.0000x reference)
"""DualMambaBlock Trainium2 kernel: 8-core SPMD Bass/Tile implementation.

Reference computes (B=4, L=256, C=32, D=128, DI=256, DS=16, DC=4, DR=8):
  T_out = temporal mamba over L (batch B*C)     -> [B,L,C,D]
  C_out = channel mamba over C (batch B*L) on gated x -> [B,L,C,D]
  gate g[b,c] = sigmoid(||mean_l(x) @ g_w + g_b||^2 / 8)

Sharding: core k handles b = k//2;  temporal: c in [16*(k%2), +16);
channel: l in [128*(k%2), +128).  Gate mean needs full L so each core also
reads the sibling half of x[b].

Device layout: everything is [d_partition(128) x tokens] ("transposed"),
tokens are (seq-major, t contiguous).  Host pre-transposes inputs and
post-transposes outputs, so all DMAs are contiguous.

v2: per-chunk pipeline (phase A of chunk n+1 overlaps ds-loop of chunk n),
causal depthwise conv fused into the input projection on the PE (4 merged
weights W_j = w_in[:, d] * conv_w[j, d], with a per-sequence-start fixup),
and a single-softplus dt (poison applied in place after dtu).
"""
import numpy as np
import ml_dtypes

import concourse.bass as bass
import concourse.bacc as bacc
import concourse.tile as tile
import concourse.mybir as mybir
from concourse.bass_utils import run_bass_kernel_spmd

F32 = mybir.dt.float32
BF16 = mybir.dt.bfloat16
AF = mybir.ActivationFunctionType
ALU = mybir.AluOpType
BF = ml_dtypes.bfloat16

B, L, C, D = 4, 256, 32, 128
DI, DS, DC, DR = 256, 16, 4, 8
ST = 4096            # tokens per core per mamba
NCH = 2              # chunks over ST
F = ST // NCH        # 2048 tokens per chunk
POISON = 40.0        # dt value whose exp(-k*dt) underflows to 0 for k>=1

_CACHE = {}
TRACE = False


def _ap3(t, p_ap, d0, d1):
    """view [128, d0(step0), d1] of a [128, d1] tile (free broadcast)."""
    return bass.AP(p_ap.tensor, p_ap.offset, [p_ap.ap[0], [0, d0], p_ap.ap[1]])


def build_program():
    nc = bacc.Bacc('TRN2', target_bir_lowering=False, debug=False, num_devices=8)

    def din(name, shape, dt=BF16):
        return nc.dram_tensor(name, shape, dt, kind='ExternalInput').ap()

    def dout(name, shape, dt=F32):
        return nc.dram_tensor(name, shape, dt, kind='ExternalOutput').ap()

    # per-core data
    xt = din('xt', [128, ST])          # temporal tokens (c-major, l contig)
    xc = din('xc', [128, ST])          # channel tokens own l-half (l-major, c contig)
    xo = din('xo', [128, ST])          # other l-half (for gate mean only)
    t_out = dout('t_out', [128, ST])
    c_out = dout('c_out', [128, ST])

    # weights (shared across cores); layouts chosen for direct DMA as lhsT
    w = {}
    for p in ('t', 'c'):
        w[p + '_w_z'] = din(p + '_w_z', [D, DI])              # z-half of w_in
        for j in range(DC):
            # merged conv-tap weights: W_j[k, d] = w_in[k, d] * conv_w[j, d]
            w[f'{p}_w_u{j}'] = din(f'{p}_w_u{j}', [D, DI])
        w[p + '_w_in_u'] = din(p + '_w_in_u', [D, DI])        # plain u-half (fixup)
        w[p + '_dteff'] = din(p + '_dteff', [DI, DI])         # lhsT [K=256, 256]
        w[p + '_w_bc'] = din(p + '_w_bc', [DI, 2 * DS])       # lhsT [K=256, 32]
        w[p + '_w_out'] = din(p + '_w_out', [DI, D])          # lhsT [K=256, 128]
        w[p + '_conv_w'] = din(p + '_conv_w', [D, 2 * DC], F32)  # [128, 8] col q*4+j
        w[p + '_conv_b'] = din(p + '_conv_b', [DI], F32)
        w[p + '_b_dt'] = din(p + '_b_dt', [DI], F32)
        w[p + '_diag_d0'] = din(p + '_diag_d0', [D, D])       # diag(d[:128])
        w[p + '_diag_d1'] = din(p + '_diag_d1', [D, D])       # diag(d[128:])
        w[p + '_a'] = din(p + '_a', [DS], F32)                # -exp(a_log[0])
    w['g_w'] = din('g_w', [D, D // 2])                        # pre-scaled by 1/L
    w['g_b'] = din('g_b', [D // 2], F32)

    with tile.TileContext(nc) as tc:
        import contextlib
        with contextlib.ExitStack() as ctx:
            wp = ctx.enter_context(tc.tile_pool(name='wp', bufs=1))
            xp = ctx.enter_context(tc.tile_pool(name='xp', bufs=1))
            big = ctx.enter_context(tc.tile_pool(name='big', bufs=2))
            work = ctx.enter_context(tc.tile_pool(name='work', bufs=4))
            sm = ctx.enter_context(tc.tile_pool(name='sm', bufs=2))
            ps_a = ctx.enter_context(tc.tile_pool(name='ps_a', bufs=2, space='PSUM'))
            ps_b = ctx.enter_context(tc.tile_pool(name='ps_b', bufs=2, space='PSUM'))
            ps_y = ctx.enter_context(tc.tile_pool(name='ps_y', bufs=1, space='PSUM'))

            # x tiles first: the gate and first matmuls wait on these
            xt_s = xp.tile([128, ST], BF16, tag='xt')
            nc.sync.dma_start(out=xt_s[:], in_=xt[:])
            xc_s = xp.tile([128, ST], BF16, tag='xc')
            nc.sync.dma_start(out=xc_s[:], in_=xc[:])
            xo_s = xp.tile([128, ST], BF16, tag='xo')
            nc.sync.dma_start(out=xo_s[:], in_=xo[:])

            # ---- load weights ------------------------------------------------
            sb = {}
            for name, ap in w.items():
                shape = list(ap.shape)
                if len(shape) == 1 and shape[0] == DI:
                    # [256] vector -> [128, 2] tile; column q holds di-tile q
                    t = wp.tile([128, 2], ap.dtype, tag='w_' + name, name='w_' + name)
                    nc.sync.dma_start(
                        out=t[:],
                        in_=bass.AP(ap.tensor, ap.offset, [[1, 128], [128, 2]]))
                elif len(shape) == 1 and shape[0] == DS:
                    # [16] vector -> broadcast across 128 partitions
                    t = wp.tile([128, DS], ap.dtype, tag='w_' + name, name='w_' + name)
                    nc.sync.dma_start(
                        out=t[:],
                        in_=bass.AP(ap.tensor, ap.offset, [[0, 128], [1, DS]]))
                elif len(shape) == 1:
                    t = wp.tile([shape[0], 1], ap.dtype, tag='w_' + name, name='w_' + name)
                    nc.sync.dma_start(out=t[:], in_=ap[:, None])
                elif shape[0] > 128:
                    # split K=256 weights into two [128, N] tiles
                    t = []
                    for kq in range(2):
                        tt = wp.tile([128, shape[1]], ap.dtype,
                                     tag=f'w_{name}_{kq}', name=f'w_{name}_{kq}')
                        nc.sync.dma_start(out=tt[:],
                                          in_=ap[kq * 128:(kq + 1) * 128, :])
                        t.append(tt)
                else:
                    t = wp.tile(shape, ap.dtype, tag='w_' + name, name='w_' + name)
                    nc.sync.dma_start(out=t[:], in_=ap[:])
                sb[name] = t

            ones1 = wp.tile([1, 128], BF16, tag='ones1')
            nc.vector.memset(ones1[:], 1.0)
            ones64 = wp.tile([64, 1], BF16, tag='ones64')
            nc.vector.memset(ones64[:], 1.0)
            from concourse.masks import make_identity
            ident = wp.tile([128, 128], BF16, tag='ident')
            make_identity(nc, ident[:])

            # ---- gate (emitted first; fills the pipeline-fill stall) --------
            def gate():
                # mean over l: view [d; c, l] of l-major tokens (col = l*32+c)
                m1 = sm.tile([128, C], F32, tag='m1')
                m2 = sm.tile([128, C], F32, tag='m2')
                nc.vector.reduce_sum(
                    m1[:], bass.AP(xc_s[:].tensor, xc_s[:].offset,
                                   [xc_s[:].ap[0], [1, C], [C, 128]]),
                    axis=mybir.AxisListType.X)
                nc.vector.reduce_sum(
                    m2[:], bass.AP(xo_s[:].tensor, xo_s[:].offset,
                                   [xo_s[:].ap[0], [1, C], [C, 128]]),
                    axis=mybir.AxisListType.X)
                msum = sm.tile([128, C], BF16, tag='msum')
                nc.vector.tensor_add(msum[:], m1[:], m2[:])
                node_ps = ps_a.tile([64, C], F32, tag='mm', name='node_ps')
                nc.tensor.matmul(node_ps[:], sb['g_w'][:], msum[:],
                                 start=True, stop=True)
                node_sq = sm.tile([64, C], BF16, tag='node_sq')
                nc.scalar.activation(node_sq[:], node_ps[:], AF.Square,
                                     bias=sb['g_b'][:], scale=1.0)
                nrm_ps = ps_a.tile([1, C], F32, tag='mm', name='nrm_ps')
                nc.tensor.matmul(nrm_ps[:], ones64[:], node_sq[:],
                                 start=True, stop=True)
                g_row = sm.tile([1, C], BF16, tag='g_row')
                nc.scalar.activation(g_row[:], nrm_ps[:], AF.Sigmoid, scale=0.125)
                grep_ps = ps_a.tile([128, C], F32, tag='mm', name='grep_ps')
                nc.tensor.matmul(grep_ps[:], ones1[:], g_row[:],
                                 start=True, stop=True)
                g_tile = sm.tile([128, C], BF16, tag='g_tile')
                nc.scalar.copy(g_tile[:], grep_ps[:])
                # xg = xc * g (broadcast over l via step-0); reuse xo slot
                xg_s = xp.tile([128, ST], BF16, tag='xo', name='xg_s')
                nc.vector.tensor_mul(
                    xg_s[:].rearrange('p (l c) -> p l c', c=C),
                    xc_s[:].rearrange('p (l c) -> p l c', c=C),
                    _ap3(g_tile, g_tile[:], L // 2, C))
                return xg_s

            # ---- mamba blocks ------------------------------------------------
            NW = F // 512                        # 512-col windows per chunk
            bc_drams = {p: nc.dram_tensor(f'{p}_bc_scratch', [2 * DS, ST],
                                          BF16).ap() for p in ('t', 'c')}

            def phase_a(pfx, xsrc, T, ch, res):
                """u/z/dt/bc for one chunk (generator: one emission step per
                yield, so the driver can interleave with a ds loop).
                Stores chunk tiles into res['tiles']."""
                nseq_ch = F // T
                base = ch * F
                w_z = sb[pfx + '_w_z']
                w_u = [sb[f'{pfx}_w_u{j}'] for j in range(DC)]
                w_in_u = sb[pfx + '_w_in_u']
                dteff = sb[pfx + '_dteff']
                w_bc = sb[pfx + '_w_bc']
                conv_w = sb[pfx + '_conv_w']
                conv_b = sb[pfx + '_conv_b']
                b_dt = sb[pfx + '_b_dt']
                bc_dram = bc_drams[pfx]

                u = [big.tile([128, F], BF16, tag=f'u{q}', name=f'u{q}')
                     for q in range(2)]
                zs = [big.tile([128, F], BF16, tag=f'zs{q}', name=f'zs{q}')
                      for q in range(2)]
                dt2 = [big.tile([128, F], BF16, tag=f'dt2{q}', name=f'dt2{q}')
                       for q in range(2)]
                dtu = [big.tile([128, F], BF16, tag=f'dtu{q}', name=f'dtu{q}')
                       for q in range(2)]
                bc = big.tile([2 * DS, F], BF16, tag='bc')

                for q in range(2):
                    qc = slice(q * 128, (q + 1) * 128)
                    for js in range(NW):
                        cols = slice(base + js * 512, base + (js + 1) * 512)
                        ocols = slice(js * 512, (js + 1) * 512)
                        # u via conv-fused projection: 4 shifted matmuls
                        up = ps_a.tile([128, 512], F32, tag='mm', name='up')
                        nc.tensor.matmul(up[:], w_u[3][:, qc],
                                         xsrc[:, cols], start=True, stop=False)
                        for j in range(3):
                            sh = 3 - j
                            lo = base + js * 512 - sh
                            last = (j == 2)
                            if lo < 0:   # chunk 0 head: clamp, fixup later
                                nc.tensor.matmul(
                                    up[:, sh:512], w_u[j][:, qc],
                                    xsrc[:, 0:512 - sh],
                                    start=False, stop=last)
                            else:
                                nc.tensor.matmul(
                                    up[:], w_u[j][:, qc],
                                    xsrc[:, lo:lo + 512],
                                    start=False, stop=last)
                        nc.scalar.activation(u[q][:, ocols], up[:], AF.Silu,
                                             bias=conv_b[:, q:q + 1], scale=1.0)
                        # z
                        zp = ps_b.tile([128, 512], F32, tag='mm', name='zp')
                        nc.tensor.matmul(zp[:], w_z[:, qc],
                                         xsrc[:, cols], start=True, stop=True)
                        nc.scalar.activation(zs[q][:, ocols], zp[:], AF.Silu)
                        yield

                    # conv fixup: first DC-1 cols of each sequence.
                    # up3 holds u_pre in [i-major] layout: col i*nseq + s.
                    nfix = 3 * nseq_ch
                    upre = ps_a.tile([128, nfix], F32, tag='mm', name='upre')
                    xa = xsrc[:]
                    x3 = bass.AP(xa.tensor, xa.offset + base,
                                 [xa.ap[0], [1, 3], [T, nseq_ch]])
                    nc.tensor.matmul(upre[:], w_in_u[:, qc], x3,
                                     start=True, stop=True)
                    up3 = work.tile([128, nfix], BF16, tag='up3', name='up3',
                                    bufs=2)
                    nc.scalar.copy(up3[:], upre[:])
                    corr = work.tile([128, nfix], BF16, tag='corr',
                                     name='corr', bufs=2)
                    ns = nseq_ch
                    cs = lambda i: corr[:, i * ns:(i + 1) * ns]
                    us_ = lambda m: up3[:, m * ns:(m + 1) * ns]
                    wj = lambda j: conv_w[:, q * 4 + j:q * 4 + j + 1]
                    # corr_i = sum_{m<=i} w[3-i+m] * u_pre[s, m]
                    nc.vector.tensor_scalar_mul(cs(0), us_(0), wj(3))
                    nc.vector.tensor_scalar_mul(cs(1), us_(0), wj(2))
                    nc.vector.scalar_tensor_tensor(
                        cs(1), us_(1), wj(3), cs(1), op0=ALU.mult, op1=ALU.add)
                    nc.vector.tensor_scalar_mul(cs(2), us_(0), wj(1))
                    nc.vector.scalar_tensor_tensor(
                        cs(2), us_(1), wj(2), cs(2), op0=ALU.mult, op1=ALU.add)
                    nc.vector.scalar_tensor_tensor(
                        cs(2), us_(2), wj(3), cs(2), op0=ALU.mult, op1=ALU.add)
                    ua = u[q][:]
                    ufix = bass.AP(ua.tensor, ua.offset,
                                   [ua.ap[0], [1, 3], [T, nseq_ch]])
                    cv = corr[:].rearrange('p (i s) -> p i s', i=3)
                    nc.scalar.activation(ufix, cv, AF.Silu,
                                         bias=conv_b[:, q:q + 1], scale=1.0)
                    yield

                # B/C compact projection [32, F] (before dt: hides the
                # bc_dram store->broadcast-load round-trip)
                for js in range(NW):
                    ocols = slice(js * 512, (js + 1) * 512)
                    bp = ps_b.tile([32, 512], F32, tag='mm', name='bp')
                    nc.tensor.matmul(bp[:], w_bc[0][:], u[0][:, ocols],
                                     start=True, stop=False)
                    nc.tensor.matmul(bp[:], w_bc[1][:], u[1][:, ocols],
                                     start=False, stop=True)
                    nc.scalar.copy(bc[:, ocols], bp[:])
                    nc.sync.dma_start(out=bc_dram[:, base + js * 512:
                                                  base + (js + 1) * 512],
                                      in_=bc[:, ocols])
                    yield

                # dt projection (K=256) + softplus (single Ln, in place)
                for q in range(2):
                    qc = slice(q * 128, (q + 1) * 128)
                    ets = []
                    for js in range(NW):
                        ocols = slice(js * 512, (js + 1) * 512)
                        dp = ps_b.tile([128, 512], F32, tag='mm', name='dp')
                        nc.tensor.matmul(dp[:], dteff[0][:, qc],
                                         u[0][:, ocols], start=True, stop=False)
                        nc.tensor.matmul(dp[:], dteff[1][:, qc],
                                         u[1][:, ocols], start=False, stop=True)
                        et = work.tile([128, 512], F32, tag='et', name='et',
                                       bufs=4)
                        nc.scalar.activation(et[:], dp[:], AF.Exp,
                                             bias=b_dt[:, q:q + 1], scale=1.0)
                        ets.append(et)
                        yield
                    for js in range(NW):
                        ocols = slice(js * 512, (js + 1) * 512)
                        nc.scalar.activation(dt2[q][:, ocols], ets[js][:],
                                             AF.Ln, bias=1.0)
                    nc.vector.tensor_mul(dtu[q][:], dt2[q][:], u[q][:])
                    # poison seq starts so exp(-k*dt2) == 0 there (scan reset)
                    nc.vector.memset(
                        dt2[q][:].rearrange('p (s t) -> p s t', t=T)[:, :, 0:1],
                        POISON)
                    yield
                res['tiles'] = {'u': u, 'zs': zs, 'dt2': dt2, 'dtu': dtu}

            def ds_block(pfx, ch, tl, step_hook=None):
                """selective-scan over DS states for one chunk -> y2 tiles."""
                base = ch * F
                a_vec = sb[pfx + '_a']
                diag_d = [sb[f'{pfx}_diag_d{q}'] for q in range(2)]
                bc_dram = bc_drams[pfx]
                u, zs, dt2, dtu = tl['u'], tl['zs'], tl['dt2'], tl['dtu']
                y2 = [big.tile([128, F], BF16, tag=f'y2_{q}', name=f'y2_{q}')
                      for q in range(2)]
                for q in range(2):
                    y_ps = ps_y.tile([128, F], F32, tag='y_ps', name='y_ps')
                    for ds in range(DS):
                        brep = work.tile([128, F], BF16, tag='brep',
                                         name='brep', bufs=4)
                        crep = work.tile([128, F], BF16, tag='crep',
                                         name='crep', bufs=4)
                        nc.sync.dma_start(
                            out=brep[:],
                            in_=bass.AP(bc_dram.tensor, ds * ST + base,
                                        [[0, 128], [1, F]]))
                        nc.sync.dma_start(
                            out=crep[:],
                            in_=bass.AP(bc_dram.tensor, (DS + ds) * ST + base,
                                        [[0, 128], [1, F]]))
                        dA = work.tile([128, F], BF16, tag='dA', name='dA',
                                       bufs=2)
                        nc.scalar.activation(dA[:], dt2[q][:], AF.Exp,
                                             scale=a_vec[:, ds:ds + 1])
                        in1 = work.tile([128, F], BF16, tag='in1', name='in1',
                                        bufs=2)
                        nc.vector.tensor_mul(in1[:], dtu[q][:], brep[:])
                        h = work.tile([128, F], BF16, tag='h', name='h',
                                      bufs=2)
                        nc.vector.tensor_tensor_scan(
                            h[:], dA[:], in1[:], 0.0,
                            op0=ALU.mult, op1=ALU.add)
                        hc = work.tile([128, F], BF16, tag='hc', name='hc',
                                       bufs=2)
                        nc.vector.tensor_mul(hc[:], h[:], crep[:])
                        for j in range(F // 512):
                            o2 = slice(j * 512, (j + 1) * 512)
                            nc.tensor.matmul(y_ps[:, o2], ident[:], hc[:, o2],
                                             start=(ds == 0), stop=False)
                        if step_hook is not None:
                            step_hook()
                    # y_ps += u * D via diag(D) matmul
                    for j in range(F // 512):
                        o2 = slice(j * 512, (j + 1) * 512)
                        nc.tensor.matmul(y_ps[:, o2], diag_d[q][:],
                                         u[q][:, o2], start=False, stop=True)
                    # y2 = y_ps * silu(z); copy PSUM->SBUF on scalar first so
                    # the DVE mul runs in 2x mode
                    yc = work.tile([128, F], BF16, tag='yc', name='yc', bufs=1)
                    nc.scalar.copy(yc[:], y_ps[:])
                    nc.vector.tensor_mul(y2[q][:], yc[:], zs[q][:])
                return y2

            def w_out_block(pfx, ch, y2, out_dram):
                base = ch * F
                w_out = sb[pfx + '_w_out']
                for js in range(NW):
                    ocols = slice(js * 512, (js + 1) * 512)
                    op = ps_a.tile([128, 512], F32, tag='mm', name='op')
                    nc.tensor.matmul(op[:], w_out[0][:], y2[0][:, ocols],
                                     start=True, stop=False)
                    nc.tensor.matmul(op[:], w_out[1][:], y2[1][:, ocols],
                                     start=False, stop=True)
                    ot = work.tile([128, 512], F32, tag='ot', name='ot',
                                   bufs=2)
                    nc.scalar.copy(ot[:], op[:])
                    nc.sync.dma_start(out=out_dram[:, base + js * 512:
                                                   base + (js + 1) * 512],
                                      in_=ot[:])

            # ---- flat 4-block pipeline --------------------------------------
            def drain(gen):
                if gen is None:
                    return
                for _ in gen:
                    pass

            xg_s = gate()
            blocks = [('t', xt_s, L, t_out, ch) for ch in range(NCH)] + \
                     [('c', xg_s, C, c_out, ch) for ch in range(NCH)]
            res0 = {}
            drain(phase_a(blocks[0][0], blocks[0][1], blocks[0][2],
                          blocks[0][4], res0))
            tiles = res0['tiles']
            for k, (pfx, xsrc, T, odram, ch) in enumerate(blocks):
                nxt = blocks[k + 1] if k + 1 < len(blocks) else None
                gen, nres = None, {}
                if nxt is not None:
                    gen = phase_a(nxt[0], nxt[1], nxt[2], nxt[4], nres)
                hook = (lambda g=gen: next(g, None)) if gen is not None else None
                y2 = ds_block(pfx, ch, tiles, step_hook=hook)
                drain(gen)
                w_out_block(pfx, ch, y2, odram)
                if nxt is not None:
                    tiles = nres['tiles']

    nc.compile()
    return nc


def _shard_host(inputs):
    """Build per-core input maps from full inputs."""
    x = np.asarray(inputs['x'], np.float32)

    def prep(pfx):
        w_in = np.asarray(inputs[pfx + 'w_in'], np.float32)
        w_xproj = np.asarray(inputs[pfx + 'w_xproj'], np.float32)
        w_dt = np.asarray(inputs[pfx + 'w_dt'], np.float32)
        dteff = w_xproj[:, :DR] @ w_dt
        conv_w = np.asarray(inputs[pfx + 'conv_w'], np.float32).reshape(DC, DI)
        conv_wd = np.ascontiguousarray(
            conv_w.reshape(DC, 2, D).transpose(2, 1, 0).reshape(D, 2 * DC))
        a_vec = -np.exp(np.asarray(inputs[pfx + 'a_log'], np.float32)[0])
        out = {
            pfx + 'w_z': w_in[:, DI:].astype(BF),
            pfx + 'w_in_u': w_in[:, :DI].astype(BF),
            pfx + 'dteff': dteff.astype(BF),
            pfx + 'w_bc': w_xproj[:, DR:].astype(BF),
            pfx + 'w_out': np.asarray(inputs[pfx + 'w_out'], np.float32).astype(BF),
            pfx + 'conv_w': conv_wd,
            pfx + 'conv_b': np.asarray(inputs[pfx + 'conv_b'], np.float32),
            pfx + 'b_dt': np.asarray(inputs[pfx + 'b_dt'], np.float32),
            pfx + 'diag_d0': np.diag(
                np.asarray(inputs[pfx + 'd'], np.float32)[:D]).astype(BF),
            pfx + 'diag_d1': np.diag(
                np.asarray(inputs[pfx + 'd'], np.float32)[D:]).astype(BF),
            pfx + 'a': a_vec,
        }
        for j in range(DC):
            out[f'{pfx}w_u{j}'] = (w_in[:, :DI] * conv_w[j][None, :]).astype(BF)
        return out

    shared = {}
    shared.update(prep('t_'))
    shared.update(prep('c_'))
    shared['g_w'] = (np.asarray(inputs['g_w_node'], np.float32) / L).astype(BF)
    shared['g_b'] = np.asarray(inputs['g_b_node'], np.float32)

    in_maps = []
    for k in range(8):
        b, half = k // 2, k % 2
        # temporal tokens: c-major within c-half -> [d, c*L + l]
        xtk = x[b, :, 16 * half:16 * (half + 1), :]          # [L, 16, D]
        xtk = np.ascontiguousarray(xtk.transpose(2, 1, 0).reshape(D, ST))
        # channel tokens own half: l-major -> [d, l*C + c]
        xch = x[b, 128 * half:128 * (half + 1)]             # [128, C, D]
        xch = np.ascontiguousarray(xch.transpose(2, 0, 1).reshape(D, ST))
        xoh = x[b, 128 * (1 - half):128 * (2 - half)]
        xoh = np.ascontiguousarray(xoh.transpose(2, 0, 1).reshape(D, ST))
        m = dict(shared)
        m['xt'] = xtk.astype(BF)
        m['xc'] = xch.astype(BF)
        m['xo'] = xoh.astype(BF)
        in_maps.append(m)
    return in_maps


def kernel(**inputs):
    if 'nc' not in _CACHE:
        _CACHE['nc'] = build_program()
    nc = _CACHE['nc']
    in_maps = _shard_host(inputs)
    res = run_bass_kernel_spmd(nc, in_maps, list(range(8)), trace=TRACE)
    _CACHE['last_result'] = res

    T_out = np.zeros((B, L, C, D), np.float32)
    C_out = np.zeros((B, L, C, D), np.float32)
    for k in range(8):
        b, half = k // 2, k % 2
        to = res.results[k]['t_out']          # [d, c*L + l]
        T_out[b, :, 16 * half:16 * (half + 1), :] = \
            to.reshape(D, 16, L).transpose(2, 1, 0)
        co = res.results[k]['c_out']          # [d, l*C + c]
        C_out[b, 128 * half:128 * (half + 1)] = \
            co.reshape(D, 128, C).transpose(1, 2, 0)
    return (T_out, C_out)
